# revision 2
# baseline (speedup 1.0000x reference)
"""Trainium2 Bass kernel for nn_Capsule (capsule attention w/ dynamic routing).

Math: in the reference, c = softmax(b, axis=1) is over a size-1 axis, so
c == 1 in every routing iteration and the module collapses to

    s[b, d] = sum_{j,e} W[0, j, d, e] * x[b, j, e]     (one big matmul)
    out     = squash(s)                                 -> (B, 1, D)

i.e. (512, 36*1024) @ (36*1024, 1024) followed by a per-row squash.

Sharding: contraction(K)-parallel over 8 NeuronCores. Each core gets
K/8 = 4608 rows of x^T and W^T (host-side layout: k-major, SBUF-tiled
[128, kt*free], bf16) and computes a partial (512, 1024) sum at the bf16
TensorEngine roofline (~61.5us of matmul). The host unshard step sums the
8 partials and applies squash. K-sharding moves ~14 MB/core from HBM vs
~151 MB/core for data-parallel (replicated weight).

Hand-scheduled raw Bass (no Tile): single interleaved pass where all 8
PSUM banks (4 b-tiles x 2 d-chunks) accumulate per k-tile, so each DMA
chunk is consumed once and the PE is the only steady-state bottleneck.

Trace-derived tuning (vs the first working version):
  * One merged semaphore per DMA chunk (W inc 16 + X inc 16, PE waits
    >=32). Safe because each sem only ever counts its own chunk's
    increments; the cross-chunk interleave hazard only applies to sems
    shared across chunks. Fewer sems shrink the end-of-NEFF per-engine
    semaphore-reset storm (~2us in the 23-sem version).
  * Last two ramp chunks merged (they landed 5-8.5us before PE consumed
    them; chunks 2-6 land with <30ns slack so the early ramp is kept).
  * Tail: bank g7 (b3, out cols 512:1023) accumulates its last 4 k-tiles
    as two 256-col regions with separate stop matmuls, so the exposed
    post-stream chain is cast(256)+64KB DMA instead of cast(512)+128KB,
    and the three final out-DMAs are spread across both HWDGE rings.
  * Chunk sems are range-cleared by the otherwise-idle GpSimd engine
    mid-tail; Sync only clears pe/cp/out at the very end.

Engine plan:
  SP  (sync):   W chunk DMAs (HWDGE), out DMAs b0/b1 + b3 quarters,
                final wait + pe/cp/out sem clear
  ACT (scalar): X chunk DMAs (HWDGE ring #2), out DMAs b2 + b3 first half
  PE  (tensor): warmup matmuls (HAM clock ramp), then the real stream;
                last 4 k-tiles run bank-major so the copy/DMA tail hides
                behind the matmul stream
  DVE (vector): PSUM -> SBUF staging copies (fp32 -> bf16 cast)
  POOL(gpsimd): mid-tail range-clear of the chunk semaphores
"""

import os
import sys
from contextlib import ExitStack

for _p in ("/opt/trn_rl_repo", "/root/.axon_site/_ro/trn_rl_repo"):
    if os.path.isdir(_p) and _p not in sys.path:
        sys.path.append(_p)

import ml_dtypes
import numpy as np

N_CAPS = 36
D = 1024
B = 512
N_CORES = 8
K = N_CAPS * D
KC = K // N_CORES
KT = KC // 128            # 36
B_TILES = B // 128        # 4
D_CHUNKS = D // 512       # 2
CHUNKS = [1, 1, 2, 2, 3, 3, 6, 6, 12]  # kt per DMA chunk (ramped)
N_WARM = 18
TAIL_KT = 4

_CACHE = {}
LAST_RESULTS = None


def _build():
    import concourse.bass as bass
    import concourse.mybir as mybir
    from concourse import bacc

    nc = bacc.Bacc("TRN2", target_bir_lowering=False, debug=False,
                   num_devices=N_CORES)
    bf16 = mybir.dt.bfloat16
    f32 = mybir.dt.float32

    # Inputs are stored chunk-major (each DMA chunk is one fully contiguous
    # HBM block) so early chunks stream at full sequential bandwidth.
    xt = nc.dram_tensor("xt", [128 * KT * B], bf16, kind="ExternalInput")
    wt = nc.dram_tensor("wt", [128 * KT * D], bf16, kind="ExternalInput")
    out = nc.dram_tensor("out", [B, D], bf16, kind="ExternalOutput")

    bounds = []
    s = 0
    for ch in CHUNKS:
        bounds.append((s, ch))
        s += ch
    assert s == KT

    with ExitStack() as ctx:
        X = ctx.enter_context(nc.sbuf_tensor("X", [128, KT * B], bf16))
        W = ctx.enter_context(nc.sbuf_tensor("W", [128, KT * D], bf16))
        scratch = ctx.enter_context(nc.sbuf_tensor("scratch", [128, 256], bf16))
        stagings = [
            ctx.enter_context(nc.sbuf_tensor(f"st{b}", [128, 1024], bf16))
            for b in range(B_TILES)
        ]
        psums = [
            ctx.enter_context(nc.psum_tensor(f"ps{g}", [128, 512], f32))
            for g in range(8)
        ]
        # One completion sem per chunk, incremented by BOTH the W DMA (16)
        # and the X DMA (16); PE waits >=32. A sem shared across *chunks*
        # would be unsafe (interleaved per-SDMA-engine increments), but a
        # sem private to one chunk only reaches 32 when both its DMAs have
        # fully landed.
        ch_sems = [ctx.enter_context(nc.semaphore(f"ch_sem{i}"))
                   for i in range(len(CHUNKS))]
        pe_sem = ctx.enter_context(nc.semaphore("pe_sem"))
        cp_sem = ctx.enter_context(nc.semaphore("cp_sem"))
        out_sem = ctx.enter_context(nc.semaphore("out_sem"))

        ch_nums = sorted(sm.num for sm in ch_sems)
        tail_nums = sorted(sm.num for sm in (pe_sem, cp_sem, out_sem))
        assert ch_nums == list(range(ch_nums[0], ch_nums[0] + len(ch_sems)))
        assert tail_nums == list(range(tail_nums[0], tail_nums[0] + 3))

        # DVE cast plan: (pe_sem threshold, psum src slice, staging dst slice,
        # cp value after). g0..g5 pair-merged per b-tile; g6 single; g7 split.
        N_OUT_DMAS = 6  # b0, b1, b2, b3d0, b3q2, b3q3

        with nc.Block(no_gpsimd_drain=True) as block:

            @block.sync
            def _(sync):
                for ci, (s0, ch) in enumerate(bounds):
                    src = wt[128 * s0 * D: 128 * (s0 + ch) * D] \
                        .rearrange("(p f) -> p f", p=128)
                    sync.dma_start(
                        out=W[:, s0 * D:(s0 + ch) * D],
                        in_=src,
                    ).then_inc(ch_sems[ci], 16)
                for b in (0, 1):
                    sync.wait_ge(cp_sem, b + 1)
                    sync.dma_start(
                        out=out[b * 128:(b + 1) * 128, :],
                        in_=stagings[b][:, :],
                    ).then_inc(out_sem, 16)
                # b3 quarters 2 and 3 (out cols 512:768, 768:1024) ride the
                # sync ring, which is idle by the tail.
                for qi in range(2):
                    sync.wait_ge(cp_sem, 5 + qi)
                    sync.dma_start(
                        out=out[3 * 128:4 * 128, 512 + qi * 256:768 + qi * 256],
                        in_=stagings[3][:, 512 + qi * 256:768 + qi * 256],
                    ).then_inc(out_sem, 16)
                sync.wait_ge(out_sem, 16 * N_OUT_DMAS)
                # zero pe/cp/out so the NEFF can re-execute (out_sem proves
                # all out DMAs completed; chunk sems were cleared by gpsimd)
                sync.sem_clear(range(tail_nums[0], tail_nums[0] + 3))

            @block.scalar
            def _(scalar):
                for ci, (s0, ch) in enumerate(bounds):
                    src = xt[128 * s0 * B: 128 * (s0 + ch) * B] \
                        .rearrange("(p f) -> p f", p=128)
                    scalar.dma_start(
                        out=X[:, s0 * B:(s0 + ch) * B],
                        in_=src,
                    ).then_inc(ch_sems[ci], 16)
                # out DMAs for b2 and b3's first half on the ACT HWDGE ring
                # (copies stay on DVE: ACT's activation-path copy is not
                # bit-exact).
                scalar.wait_ge(cp_sem, 3)
                scalar.dma_start(
                    out=out[2 * 128:3 * 128, :],
                    in_=stagings[2][:, :],
                ).then_inc(out_sem, 16)
                scalar.wait_ge(cp_sem, 4)
                scalar.dma_start(
                    out=out[3 * 128:4 * 128, 0:512],
                    in_=stagings[3][:, 0:512],
                ).then_inc(out_sem, 16)

            @block.tensor
            def _(tensor):
                # warm the PE clock while DMAs stream (results discarded)
                for _i in range(N_WARM):
                    tensor.matmul(psums[7][:, 0:256], lhsT=scratch[:, 0:128],
                                  rhs=scratch[:, :], start=True, stop=True)

                def mm_for(kt, b, dd):
                    g = b * 2 + dd
                    if g == 7 and kt >= KT - TAIL_KT:
                        return  # emitted as split 256-col regions in the tail
                    tensor.matmul(
                        psums[g][:, :],
                        lhsT=X[:, kt * B + b * 128: kt * B + (b + 1) * 128],
                        rhs=W[:, kt * D + dd * 512: kt * D + (dd + 1) * 512],
                        start=(kt == 0),
                        stop=False,
                        skip_group_check=(g == 7),
                    )

                # kt-major over kt 0..KT-5 (tracks DMA chunk arrival), then
                # bank-major for the last 4 k-tiles so early banks finish
                # early and the DVE copy chain + out-DMA receipts hide
                # behind the matmul tail. Per-boundary chunk waits: the DMA
                # stream runs only marginally ahead of PE consumption
                # (supply-limited steady state), so each chunk must be
                # awaited at its own boundary (hoisting them early measured
                # 15-20us SLOWER).
                chunk_idx = 0
                next_boundary = 0
                for kt in range(KT - TAIL_KT):
                    if kt == next_boundary:
                        tensor.wait_ge(ch_sems[chunk_idx], 32)
                        next_boundary += CHUNKS[chunk_idx]
                        chunk_idx += 1
                    for b in range(B_TILES):
                        for dd in range(D_CHUNKS):
                            mm_for(kt, b, dd)
                while chunk_idx < len(CHUNKS):
                    tensor.wait_ge(ch_sems[chunk_idx], 32)
                    chunk_idx += 1
                # tail: banks g0..g6 full-width, then g7 as two 256-col
                # regions (a: psum cols 0:256 <-> out cols 512:768, b: psum
                # cols 256:512 <-> out cols 768:1024) so the last exposed
                # cast+DMA works on 64KB quarters.
                for g in range(7):
                    b, dd = divmod(g, 2)
                    for kt in range(KT - TAIL_KT, KT):
                        mm = tensor.matmul(
                            psums[g][:, :],
                            lhsT=X[:, kt * B + b * 128: kt * B + (b + 1) * 128],
                            rhs=W[:, kt * D + dd * 512: kt * D + (dd + 1) * 512],
                            start=False,
                            stop=(kt == KT - 1),
                        )
                        if kt == KT - 1:
                            mm.then_inc(pe_sem, 1)
                for half in range(2):
                    for kt in range(KT - TAIL_KT, KT):
                        mm = tensor.matmul(
                            psums[7][:, half * 256:(half + 1) * 256],
                            lhsT=X[:, kt * B + 3 * 128: kt * B + 4 * 128],
                            rhs=W[:, kt * D + 512 + half * 256:
                                  kt * D + 768 + half * 256],
                            start=False,
                            stop=(kt == KT - 1),
                            skip_group_check=True,
                        )
                        if kt == KT - 1:
                            mm.then_inc(pe_sem, 1)

            @block.vector
            def _(vector):
                # b0..b2: one wait per b-tile, two casts, one cp inc
                for b in range(3):
                    vector.wait_ge(pe_sem, 2 * (b + 1))
                    vector.tensor_copy(
                        stagings[b][:, 0:512], psums[2 * b][:, :])
                    vector.tensor_copy(
                        stagings[b][:, 512:1024], psums[2 * b + 1][:, :],
                    ).then_inc(cp_sem, 1)
                # b3 first half (bank g6)
                vector.wait_ge(pe_sem, 7)
                vector.tensor_copy(
                    stagings[3][:, 0:512], psums[6][:, :],
                ).then_inc(cp_sem, 1)
                # b3 quarters (bank g7 split regions)
                for half in range(2):
                    vector.wait_ge(pe_sem, 8 + half)
                    vector.tensor_copy(
                        stagings[3][:, 512 + half * 256:768 + half * 256],
                        psums[7][:, half * 256:(half + 1) * 256],
                    ).then_inc(cp_sem, 1)

            @block.gpsimd
            def _(gpsimd):
                # All chunk sems are final once the last bank's accumulation
                # closes (input DMAs finished long before); clear them here
                # so the end-of-NEFF path only handles pe/cp/out.
                gpsimd.wait_ge(pe_sem, 7)
                gpsimd.sem_clear(range(ch_nums[0], ch_nums[0] + len(ch_sems)))

    nc.compile()
    return nc


def _get_nc():
    if "nc" not in _CACHE:
        _CACHE["nc"] = _build()
    return _CACHE["nc"]


def _chunk_major(a, cols):
    """[N_CORES, 128, KT*cols] -> [N_CORES, 128*KT*cols] with each DMA
    chunk's [128, ch*cols] block stored contiguously (kernel reads chunk ci
    at flat offset 128*s0*cols)."""
    n = a.shape[0]
    flat = np.empty((n, 128 * KT * cols), dtype=a.dtype)
    s = 0
    for ch in CHUNKS:
        blk = a[:, :, s * cols:(s + ch) * cols]
        flat[:, 128 * s * cols:128 * (s + ch) * cols] = blk.reshape(n, -1)
        s += ch
    return flat


def _shard_inputs(x, weight):
    bf16 = ml_dtypes.bfloat16
    xT = np.ascontiguousarray(np.transpose(x, (1, 2, 0))).reshape(K, B)
    xts = (xT.reshape(N_CORES, KT, 128, B)
              .transpose(0, 2, 1, 3)
              .reshape(N_CORES, 128, KT * B)
              .astype(bf16))
    wk = np.ascontiguousarray(np.transpose(weight[0], (0, 2, 1))).reshape(K, D)
    wts = (wk.reshape(N_CORES, KT, 128, D)
              .transpose(0, 2, 1, 3)
              .reshape(N_CORES, 128, KT * D)
              .astype(bf16))
    return _chunk_major(xts, B), _chunk_major(wts, D)


def _ensure_trace_shim():
    """If the environment requests NTFF tracing (BASS_TRACE=1) but this
    container's antenv lacks axon_hooks, provide it from trn_boot's ctypes
    implementation so run_bass_kernel_spmd doesn't crash mid-trace."""
    try:
        import antenv.axon_hooks  # noqa: F401
        return
    except ImportError:
        pass
    try:
        import types

        import antenv
        import trn_agent_boot.trn_boot as tb
        from concourse import bass_utils

        hook = tb._ntff_profile_via_ctypes("/opt/axon/libaxon_pjrt.so")
        mod = types.ModuleType("antenv.axon_hooks")
        mod.get_axon_ntff_profile_hook = lambda: hook
        mod.set_axon_ntff_profile_hook = lambda h: None
        antenv.axon_hooks = mod
        sys.modules["antenv.axon_hooks"] = mod
        if not getattr(bass_utils.upload_artifacts, "_patched", False):
            bass_utils.upload_artifacts = lambda tmpdir: tmpdir
            bass_utils.upload_artifacts._patched = True
    except Exception:
        # tracing unavailable -> disable rather than crash the run
        os.environ["BASS_NEVER_TRACE"] = "1"


def kernel(x, weight, isLastLayer=None):
    global LAST_RESULTS
    _ensure_trace_shim()
    from concourse.bass_utils import run_bass_kernel_spmd

    x = np.asarray(x, dtype=np.float32)
    weight = np.asarray(weight, dtype=np.float32)

    xts, wts = _shard_inputs(x, weight)
    in_maps = [{"xt": np.ascontiguousarray(xts[i]),
                "wt": np.ascontiguousarray(wts[i])} for i in range(N_CORES)]

    nc = _get_nc()
    res = run_bass_kernel_spmd(nc, in_maps, core_ids=list(range(N_CORES)))
    LAST_RESULTS = res

    s = np.zeros((B, D), dtype=np.float32)
    for core_out in res.results:
        s += np.asarray(core_out["out"]).astype(np.float32)
    norm = np.sqrt((s.astype(np.float64) ** 2).sum(axis=-1, keepdims=True)).astype(np.float32)
    scale = norm ** 2 / (1.0 + norm ** 2) / (norm + 1e-8)
    return (scale * s)[:, None, :].astype(np.float32)
